# revision 1
# baseline (speedup 1.0000x reference)
"""DeformConv2d (B=8, C=128, H=W=64, K=3x3, pad 1, stride 1) on 8 trn2 NeuronCores.

Data-parallel over batch: core b handles image b. Per core:
  - x image zero-padded with a 2-pixel ring (rows/cols -2..65) so the
    reference's out-of-bounds corner masking is exactly reproduced by the
    padding (clamped sample indices land in the zero ring).
  - P2[i] packs (bf16(XP[i]), bf16(XP[i+68])) into one fp32 word, so a single
    gpsimd ap_gather index fetches the vertical corner pair (y0,x)/(y0+1,x).
    A second gather at lin+1 fetches the (x0+1) pair: all 4 bilinear corners
    in 2 gathers per kernel tap.
  - Bilinear weights (pure fraction products; no masks needed) are computed
    on DVE in a [128, 288] layout (position%128 on partitions), staged to
    DRAM, and broadcast to all 128 partitions with a stride-0-source DMA.
  - Per (quarter, tap): weighted corner products in bf16 (DVE 2x mode), then
    PE matmuls accumulate both corner-pair lanes and all 9 taps into PSUM.
  - Tail per quarter: even+odd PSUM lanes + bias -> fp32 output.
"""
import numpy as np
import ml_dtypes

B, CIN, H, W = 8, 128, 64, 64
COUT, KH, KW = 128, 3, 3
K = KH * KW
HO, WO = 64, 64
P = 128                      # partitions
NPOS = HO * WO               # 4096 output positions per image
Q = NPOS // P                # 32 free-dim columns in the [128, 288] gen layout
PADR = 2                     # zero-pad ring width
HP = H + 2 * PADR            # 68
WP = W + 2 * PADR            # 68
NE = HP * WP                 # 4624 padded elements
NXP = NE + WP + 1            # XP alloc with tail zeros for corner shifts
NQT = NPOS // 4              # 1024 positions per PSUM quarter
QI = NQT // 16               # 64 idx-cols per quarter
FB = 1024.0                  # floor-trick bias constant


def _build_kernel(repeat=1):
    import concourse.bacc as bacc
    import concourse.mybir as mybir
    import concourse.tile as tile
    import concourse.library_config as library_config

    nc = bacc.Bacc("TRN2", target_bir_lowering=False, debug=False, num_devices=8)
    f32, bf16, i16 = mybir.dt.float32, mybir.dt.bfloat16, mybir.dt.int16
    ALU = mybir.AluOpType

    x_d = nc.dram_tensor("x", [P, NPOS], f32, kind="ExternalInput")
    off_d = nc.dram_tensor("offs", [2 * K, NPOS], f32, kind="ExternalInput")
    wmat_d = nc.dram_tensor("wmat", [P, K * COUT], bf16, kind="ExternalInput")
    bias_d = nc.dram_tensor("bias", [P, 1], f32, kind="ExternalInput")
    hob_d = nc.dram_tensor("hob", [P, K * Q], f32, kind="ExternalInput")
    wob_d = nc.dram_tensor("wob", [P, K * Q], f32, kind="ExternalInput")
    out_d = nc.dram_tensor("out", [P, NPOS], f32, kind="ExternalOutput")

    with tile.TileContext(nc) as tc:
        with tc.tile_pool(name="const", bufs=1) as cpool, \
             tc.tile_pool(name="gen", bufs=1) as gpool, \
             tc.tile_pool(name="wbc", bufs=3) as wpool, \
             tc.tile_pool(name="gath", bufs=3) as gapool, \
             tc.tile_pool(name="mm", bufs=3) as mpool, \
             tc.tile_pool(name="outp", bufs=2) as opool, \
             tc.tile_pool(name="dramw", bufs=1, space="DRAM") as dpool, \
             tc.tile_pool(name="ps", bufs=1, space="PSUM") as pspool:

            # staging for weight rows: [k, 4*NPOS] 4-lane interleaved, natural p
            wrow = dpool.tile([K, 4 * NPOS], mybir.dt.bfloat16)

            nc.gpsimd.load_library(library_config.ap_gather)

            for _rep in range(repeat):
              # -------------- stage 0: loads + padded image + P2 pack -------
              XP = cpool.tile([P, NXP], f32)
              nc.vector.memset(XP[:], 0.0)
              # x rows into the padded interior (strided dst AP, one DMA)
              xp_img = XP[:, 0:NE].rearrange("p (h w) -> p h w", h=HP, w=WP)
              nc.sync.dma_start(
                  out=xp_img[:, PADR : PADR + H, PADR : PADR + W],
                  in_=x_d.ap().rearrange("p (h w) -> p h w", h=H, w=W),
              )

              wmat = cpool.tile([P, K * COUT], bf16)
              nc.sync.dma_start(out=wmat[:], in_=wmat_d.ap())
              bias = cpool.tile([P, 1], f32)
              nc.sync.dma_start(out=bias[:], in_=bias_d.ap())
              hob = cpool.tile([P, K * Q], f32)
              nc.sync.dma_start(out=hob[:], in_=hob_d.ap())
              wob = cpool.tile([P, K * Q], f32)
              nc.sync.dma_start(out=wob[:], in_=wob_d.ap())
              # permuted offsets: offy[Pp, k*Q+q] = off[2k, q*128+Pp], offx likewise
              offy = cpool.tile([P, K * Q], f32)
              offx = cpool.tile([P, K * Q], f32)
              for k in range(K):
                  nc.scalar.dma_start(
                      out=offy[:, k * Q : (k + 1) * Q],
                      in_=off_d.ap()[2 * k].rearrange("(q p) -> p q", p=P),
                  )
                  nc.scalar.dma_start(
                      out=offx[:, k * Q : (k + 1) * Q],
                      in_=off_d.ap()[2 * k + 1].rearrange("(q p) -> p q", p=P),
                  )

              # P4 pack: 4 bf16 corner lanes per index in two fp32 words:
              # lanes (XP[i], XP[i+WP], XP[i+1], XP[i+WP+1]) = (A, C, B, D)
              P4 = cpool.tile([P, 2 * NE], f32)
              p4h = P4[:].bitcast(mybir.dt.bfloat16)  # [P, 4*NE]
              nc.scalar.copy(out=p4h[:, 0 : 4 * NE : 4], in_=XP[:, 0:NE])
              nc.scalar.copy(out=p4h[:, 1 : 4 * NE : 4], in_=XP[:, WP : NE + WP])
              nc.scalar.copy(out=p4h[:, 2 : 4 * NE : 4], in_=XP[:, 1 : NE + 1])
              nc.scalar.copy(out=p4h[:, 3 : 4 * NE : 4],
                             in_=XP[:, WP + 1 : NE + WP + 1])

              # ---------------- stage 1: weights + indices -------------------
              NG = K * Q  # 288
              pyb = gpool.tile([P, NG], f32)
              pxb = gpool.tile([P, NG], f32)
              # pyb = (offy + FB) + hob   (hob already holds ho - 1 + ky)
              nc.vector.scalar_tensor_tensor(
                  out=pyb[:], in0=offy[:], scalar=FB, in1=hob[:],
                  op0=ALU.add, op1=ALU.add)
              nc.vector.scalar_tensor_tensor(
                  out=pxb[:], in0=offx[:], scalar=FB, in1=wob[:],
                  op0=ALU.add, op1=ALU.add)
              # floor robust to cast rounding mode (trunc in sim, RN on hw):
              # y0 = cast(pyb); lyr = pyb - y0; adj = (lyr < 0); floor = y0 - adj
              def floor_frac(pb, sfx):
                  i0 = gpool.tile([P, NG], mybir.dt.int32, tag="ffi" + sfx)
                  nc.vector.tensor_copy(out=i0[:], in_=pb[:])
                  f0 = gpool.tile([P, NG], f32, tag="fff" + sfx)
                  nc.vector.tensor_copy(out=f0[:], in_=i0[:])
                  lr = gpool.tile([P, NG], f32, tag="ffl" + sfx)
                  nc.vector.tensor_tensor(out=lr[:], in0=pb[:], in1=f0[:],
                                          op=ALU.subtract)
                  adj = gpool.tile([P, NG], f32, tag="ffa" + sfx)
                  nc.vector.tensor_scalar(out=adj[:], in0=lr[:], scalar1=0.0,
                                          scalar2=None, op0=ALU.is_lt)
                  fr = gpool.tile([P, NG], f32, tag="ffr" + sfx)
                  nc.vector.tensor_tensor(out=fr[:], in0=lr[:], in1=adj[:],
                                          op=ALU.add)
                  fl = gpool.tile([P, NG], f32, tag="ffo" + sfx)
                  nc.vector.tensor_tensor(out=fl[:], in0=f0[:], in1=adj[:],
                                          op=ALU.subtract)
                  return fl, fr
              y0f, ly = floor_frac(pyb, "y")
              x0f, lx = floor_frac(pxb, "x")
              omly = gpool.tile([P, NG], f32)
              omlx = gpool.tile([P, NG], f32)
              nc.vector.tensor_scalar(out=omly[:], in0=ly[:], scalar1=-1.0, scalar2=1.0,
                                      op0=ALU.mult, op1=ALU.add)
              nc.vector.tensor_scalar(out=omlx[:], in0=lx[:], scalar1=-1.0, scalar2=1.0,
                                      op0=ALU.mult, op1=ALU.add)
              # clamp biased corner coords to [-PADR, 64]+FB
              ycl = gpool.tile([P, NG], f32)
              xcl = gpool.tile([P, NG], f32)
              nc.vector.tensor_scalar(out=ycl[:], in0=y0f[:], scalar1=FB - PADR,
                                      scalar2=FB + 64.0, op0=ALU.max, op1=ALU.min)
              nc.vector.tensor_scalar(out=xcl[:], in0=x0f[:], scalar1=FB - PADR,
                                      scalar2=FB + 64.0, op0=ALU.max, op1=ALU.min)
              # lin = (ycl-FB+PADR)*WP + (xcl-FB+PADR) = WP*ycl + xcl - (WP+1)*(FB-PADR)
              linf = gpool.tile([P, NG], f32)
              nc.vector.scalar_tensor_tensor(
                  out=linf[:], in0=ycl[:], scalar=float(WP), in1=xcl[:],
                  op0=ALU.mult, op1=ALU.add)
              linf2 = gpool.tile([P, NG], f32)
              nc.vector.tensor_scalar(out=linf2[:], in0=linf[:],
                                      scalar1=-(WP + 1.0) * (FB - PADR),
                                      scalar2=None, op0=ALU.add)
              lin16 = gpool.tile([P, NG], i16)
              nc.vector.tensor_copy(out=lin16[:], in_=linf2[:])

              # weight products, 4-lane interleave matching P4 lane order
              wpre_cat = gpool.tile([P, 4 * NG], bf16)
              wv = wpre_cat[:].rearrange("p (k q j) -> p k q j",
                                         k=K, q=Q, j=4)
              omly3 = omly[:].rearrange("p (k q) -> p k q", k=K, q=Q)
              ly3 = ly[:].rearrange("p (k q) -> p k q", k=K, q=Q)
              omlx3 = omlx[:].rearrange("p (k q) -> p k q", k=K, q=Q)
              lx3 = lx[:].rearrange("p (k q) -> p k q", k=K, q=Q)
              nc.vector.tensor_tensor(out=wv[:, :, :, 0], in0=omly3, in1=omlx3,
                                      op=ALU.mult)  # w00 (A)
              nc.vector.tensor_tensor(out=wv[:, :, :, 1], in0=ly3, in1=omlx3,
                                      op=ALU.mult)  # w10 (C)
              nc.vector.tensor_tensor(out=wv[:, :, :, 2], in0=omly3, in1=lx3,
                                      op=ALU.mult)  # w01 (B)
              nc.vector.tensor_tensor(out=wv[:, :, :, 3], in0=ly3, in1=lx3,
                                      op=ALU.mult)  # w11 (D)
              # stage to DRAM: wrow[k, (q*128+Pp)*4 + j] = wpre_cat[Pp, kqj]
              wrow_v = wrow[:].rearrange(
                  "k (q p j) -> p k q j", k=K, q=Q, p=P, j=4)
              nc.sync.dma_start(out=wrow_v, in_=wpre_cat[:])

              # index tensors: wrapped-16 layout for ap_gather
              # idxw[16g + r, k*(8Q) + 8q + u] = lin16[16u + r, k*Q + q]
              NI = 8 * K * Q  # 2304 idx-cols total (256 per tap)
              idxw = gpool.tile([P, NI], i16)
              for u in range(8):
                  nc.scalar.dma_start(
                      out=idxw[0:16, :].rearrange(
                          "p (k q u) -> p k q u", k=K, q=Q, u=8)[:, :, :, u],
                      in_=lin16[16 * u : 16 * u + 16, :].rearrange(
                          "p (k q) -> p k q", k=K, q=Q),
                  )
              for g in range(1, 8):
                  nc.scalar.dma_start(out=idxw[16 * g : 16 * g + 16, :],
                                      in_=idxw[0:16, :])


              # ---------------- stage 2+3: gather/mul/matmul per quarter -----
              for qt in range(4):
                  psum = pspool.tile([P, 4 * NQT], f32, tag="ps")
                  for k in range(K):
                      # broadcast this (tap, quarter) 4-lane weight slice
                      wbc = wpool.tile([P, 4 * NQT], bf16, tag="wb")
                      sl = slice(qt * 4 * NQT, (qt + 1) * 4 * NQT)
                      nc.sync.dma_start(
                          out=wbc[:],
                          in_=wrow[k : k + 1, sl].to_broadcast((P, 4 * NQT)))
                      i0 = k * 4 * QI + qt * QI
                      g4 = gapool.tile([P, 2 * NQT], f32, tag="g")
                      nc.gpsimd.ap_gather(
                          g4[:], P4[:], idxw[:, i0 : i0 + QI],
                          channels=P, num_elems=NE, d=2, num_idxs=NQT)
                      m = mpool.tile([P, 4 * NQT], bf16, tag="m")
                      nc.vector.tensor_tensor(
                          out=m[:],
                          in0=g4[:].bitcast(mybir.dt.bfloat16),
                          in1=wbc[:], op=ALU.mult)
                      lhsT = wmat[:, k * COUT : (k + 1) * COUT]
                      for bk in range(8):  # 512-col pieces, one PSUM bank each
                          c0 = bk * 512
                          nc.tensor.matmul(
                              psum[:, c0 : c0 + 512], lhsT,
                              m[:, c0 : c0 + 512],
                              start=(k == 0), stop=(k == K - 1),
                              skip_group_check=True)
                  # tail: sum 4 corner lanes + bias -> fp32 out
                  pv = psum[:].rearrange("p (n j) -> p n j", j=4)
                  t = opool.tile([P, NQT], f32, tag="t")
                  nc.vector.tensor_scalar(
                      out=t[:], in0=pv[:, :, 0],
                      scalar1=bias[:, 0:1], scalar2=None, op0=ALU.add)
                  t2 = opool.tile([P, NQT], f32, tag="t2")
                  nc.vector.tensor_tensor(
                      out=t2[:], in0=t[:], in1=pv[:, :, 1], op=ALU.add)
                  t3 = opool.tile([P, NQT], f32, tag="t3")
                  nc.vector.tensor_tensor(
                      out=t3[:], in0=t2[:], in1=pv[:, :, 2], op=ALU.add)
                  o = opool.tile([P, NQT], f32, tag="o")
                  nc.vector.tensor_tensor(
                      out=o[:], in0=t3[:], in1=pv[:, :, 3], op=ALU.add)
                  nc.sync.dma_start(
                      out=out_d.ap()[:, qt * NQT : (qt + 1) * NQT], in_=o[:])

    nc.compile()
    return nc


_NC_CACHE = None


def _host_inputs(x, offset, weight, bias):
    """Per-core input maps (core b <- batch b) + replicated constants."""
    wq = np.ascontiguousarray(weight, np.float32)  # [COUT, CIN, KH, KW]
    # wmat[c, k*COUT + o] = weight[o, c, ky, kx]
    wmat = wq.reshape(COUT, CIN, K).transpose(1, 2, 0).reshape(CIN, K * COUT)
    wmat = np.ascontiguousarray(wmat).astype(ml_dtypes.bfloat16)
    bias_h = np.ascontiguousarray(bias, np.float32).reshape(P, 1)
    # hob[Pp, k*Q+q] = ho(p) - 1 + ky,  wob = wo(p) - 1 + kx,  p = q*128 + Pp
    p_of = (np.arange(Q)[:, None] * P + np.arange(P)[None, :])  # [Q, P]
    ho = (p_of // WO).astype(np.float32)
    wo = (p_of % WO).astype(np.float32)
    hob = np.empty((P, K * Q), np.float32)
    wob = np.empty((P, K * Q), np.float32)
    for k in range(K):
        hob[:, k * Q : (k + 1) * Q] = (ho + (k // 3 - 1)).T
        wob[:, k * Q : (k + 1) * Q] = (wo + (k % 3 - 1)).T
    in_maps = []
    for b in range(B):
        in_maps.append({
            "x": np.ascontiguousarray(x[b], np.float32).reshape(P, NPOS),
            "offs": np.ascontiguousarray(offset[b], np.float32).reshape(2 * K, NPOS),
            "wmat": wmat,
            "bias": bias_h,
            "hob": hob,
            "wob": wob,
        })
    return in_maps


def kernel(x, offset, weight, bias):
    global _NC_CACHE
    from concourse.bass_utils import run_bass_kernel_spmd

    if _NC_CACHE is None:
        _NC_CACHE = _build_kernel()
    nc = _NC_CACHE
    in_maps = _host_inputs(x, offset, weight, bias)
    res = run_bass_kernel_spmd(nc, in_maps, list(range(B)))
    out = np.stack([res.results[b]["out"].reshape(COUT, HO, WO) for b in range(B)])
    return out.astype(np.float32)


if __name__ == "__main__":
    import sys
    d = np.load("/tmp/inputs.npz")
    if len(sys.argv) > 1 and sys.argv[1] == "sim":
        from concourse.bass_interp import CoreSim
        nc = _build_kernel()
        in_maps = _host_inputs(d["x"], d["offset"], d["weight"], d["bias"])
        sim = CoreSim(nc)
        for kk, vv in in_maps[0].items():
            sim.tensor(kk)[:] = vv
        sim.simulate()
        out = np.asarray(sim.tensor("out")).reshape(1, COUT, HO, WO)
        exp = np.load("/tmp/expected.npy")[0:1]
    else:
        out = kernel(d["x"], d["offset"], d["weight"], d["bias"])
        exp = np.load("/tmp/expected.npy")
    err = np.abs(out - exp)
    print("rel l2:", np.linalg.norm(out - exp) / np.linalg.norm(exp))
    print("absmax rel:", err.max() / np.abs(exp).max())



# revision 2
# speedup vs baseline: 1.0043x; 1.0043x over previous
"""DeformConv2d (B=8, C=128, H=W=64, K=3x3, pad 1, stride 1) on 8 trn2 NeuronCores.

Data-parallel over batch: core b handles image b. Per core:
  - Host packs x into P4: for each padded pixel i, one fp32 word pair holding
    bf16 lanes (XP[i], XP[i+68], XP[i+1], XP[i+69]) -- the 2x2 bilinear patch.
    One gpsimd ap_gather index fetches all 4 corners (d=2 fp32 words).
  - Host pre-transposes offsets to [128, 2*K*Q] (position%128 on partitions)
    so index/weight math runs directly on DVE with zero transpose DMAs.
  - Bilinear corner weights (4 bf16 lanes per tap-position) are computed on
    DVE compactly, staged per-tap to DRAM rows, and broadcast to all 128
    partitions with stride-0-source DMAs round-robined over the two HWDGE
    rings (sync/scalar), 512KB per (eighth, tap).
  - Per (eighth, tap): gather (gpsimd), weight multiply (DVE, bf16 2x),
    4x 512-col matmuls (PE) accumulating 9 taps into a 4-bank PSUM tile.
  - Tail per eighth: sum 4 corner lanes + bias -> fp32 out, stored via the
    gpsimd SWDGE ring to keep HWDGE rings free for weight broadcasts.
"""
import numpy as np
import ml_dtypes

B, CIN, H, W = 8, 128, 64, 64
COUT, KH, KW = 128, 3, 3
K = KH * KW
HO, WO = 64, 64
P = 128                      # partitions
NPOS = HO * WO               # 4096 output positions per image
Q = NPOS // P                # 32 position-blocks of 128
PADR = 2                     # zero-pad ring width
HP = H + 2 * PADR            # 68
WP = W + 2 * PADR            # 68
NE = HP * WP                 # 4624 padded elements
NXP = NE + WP + 1            # padded alloc with tail zeros for corner shifts
NE8 = 8                      # eighths
NQ8 = NPOS // NE8            # 512 positions per eighth
QI8 = NQ8 // 16              # 32 idx-cols per eighth per tap
FB = 1024.0                  # floor-trick bias constant


def _build_kernel():
    import concourse.bacc as bacc
    import concourse.mybir as mybir
    import concourse.tile as tile
    import concourse.library_config as library_config

    nc = bacc.Bacc("TRN2", target_bir_lowering=False, debug=False, num_devices=8)
    f32, bf16, i16 = mybir.dt.float32, mybir.dt.bfloat16, mybir.dt.int16
    ALU = mybir.AluOpType

    p4_d = nc.dram_tensor("p4", [P, 2 * NE], f32, kind="ExternalInput")
    offt_d = nc.dram_tensor("offt", [P, 2 * K * Q], f32, kind="ExternalInput")
    hw_d = nc.dram_tensor("hobwob", [P, 2 * K * Q], f32, kind="ExternalInput")
    wmat_d = nc.dram_tensor("wmat", [P, K * COUT], bf16, kind="ExternalInput")
    bias_d = nc.dram_tensor("bias", [P, 1], f32, kind="ExternalInput")
    out_d = nc.dram_tensor("out", [P, NPOS], f32, kind="ExternalOutput")

    NG = K * Q  # 288

    with tile.TileContext(nc) as tc:
        with tc.tile_pool(name="const", bufs=1) as cpool, \
             tc.tile_pool(name="gen", bufs=1) as gpool, \
             tc.tile_pool(name="wbc", bufs=4) as wpool, \
             tc.tile_pool(name="gath", bufs=4) as gapool, \
             tc.tile_pool(name="mm", bufs=3) as mpool, \
             tc.tile_pool(name="outp", bufs=2) as opool, \
             tc.tile_pool(name="dramw", bufs=1, space="DRAM") as dpool, \
             tc.tile_pool(name="ps", bufs=2, space="PSUM") as pspool:

            wrow = dpool.tile([K, 4 * NPOS], mybir.dt.bfloat16)

            nc.gpsimd.load_library(library_config.ap_gather)

            # ---------------- input loads ------------------------------
            offt = cpool.tile([P, 2 * NG], f32)
            nc.sync.dma_start(out=offt[:], in_=offt_d.ap())
            hw = cpool.tile([P, 2 * NG], f32)
            nc.scalar.dma_start(out=hw[:], in_=hw_d.ap())
            P4 = cpool.tile([P, 2 * NE], f32)
            nc.sync.dma_start(out=P4[:, 0:NE], in_=p4_d.ap()[:, 0:NE])
            nc.scalar.dma_start(out=P4[:, NE:2 * NE], in_=p4_d.ap()[:, NE:2 * NE])
            wmat = cpool.tile([P, K * COUT], bf16)
            nc.scalar.dma_start(out=wmat[:], in_=wmat_d.ap())
            bias = cpool.tile([P, 1], f32)
            nc.sync.dma_start(out=bias[:], in_=bias_d.ap())

            # ---------------- index + weight math (compact) ------------
            pyb = gpool.tile([P, NG], f32)
            pxb = gpool.tile([P, NG], f32)
            nc.vector.scalar_tensor_tensor(
                out=pyb[:], in0=offt[:, 0:NG], scalar=FB, in1=hw[:, 0:NG],
                op0=ALU.add, op1=ALU.add)
            nc.vector.scalar_tensor_tensor(
                out=pxb[:], in0=offt[:, NG:2 * NG], scalar=FB, in1=hw[:, NG:2 * NG],
                op0=ALU.add, op1=ALU.add)

            # floor robust to cast rounding mode (trunc in sim, RN on hw)
            def floor_frac(pb, sfx):
                i0 = gpool.tile([P, NG], mybir.dt.int32, tag="ffi" + sfx)
                nc.vector.tensor_copy(out=i0[:], in_=pb[:])
                f0 = gpool.tile([P, NG], f32, tag="fff" + sfx)
                nc.vector.tensor_copy(out=f0[:], in_=i0[:])
                lr = gpool.tile([P, NG], f32, tag="ffl" + sfx)
                nc.vector.tensor_tensor(out=lr[:], in0=pb[:], in1=f0[:],
                                        op=ALU.subtract)
                adj = gpool.tile([P, NG], f32, tag="ffa" + sfx)
                nc.vector.tensor_scalar(out=adj[:], in0=lr[:], scalar1=0.0,
                                        scalar2=None, op0=ALU.is_lt)
                fr = gpool.tile([P, NG], f32, tag="ffr" + sfx)
                nc.vector.tensor_tensor(out=fr[:], in0=lr[:], in1=adj[:],
                                        op=ALU.add)
                fl = gpool.tile([P, NG], f32, tag="ffo" + sfx)
                nc.vector.tensor_tensor(out=fl[:], in0=f0[:], in1=adj[:],
                                        op=ALU.subtract)
                return fl, fr

            y0f, ly = floor_frac(pyb, "y")
            x0f, lx = floor_frac(pxb, "x")
            omly = gpool.tile([P, NG], f32)
            omlx = gpool.tile([P, NG], f32)
            nc.vector.tensor_scalar(out=omly[:], in0=ly[:], scalar1=-1.0,
                                    scalar2=1.0, op0=ALU.mult, op1=ALU.add)
            nc.vector.tensor_scalar(out=omlx[:], in0=lx[:], scalar1=-1.0,
                                    scalar2=1.0, op0=ALU.mult, op1=ALU.add)
            ycl = gpool.tile([P, NG], f32)
            xcl = gpool.tile([P, NG], f32)
            nc.vector.tensor_scalar(out=ycl[:], in0=y0f[:], scalar1=FB - PADR,
                                    scalar2=FB + 64.0, op0=ALU.max, op1=ALU.min)
            nc.vector.tensor_scalar(out=xcl[:], in0=x0f[:], scalar1=FB - PADR,
                                    scalar2=FB + 64.0, op0=ALU.max, op1=ALU.min)
            linf = gpool.tile([P, NG], f32)
            nc.vector.scalar_tensor_tensor(
                out=linf[:], in0=ycl[:], scalar=float(WP), in1=xcl[:],
                op0=ALU.mult, op1=ALU.add)
            linf2 = gpool.tile([P, NG], f32)
            nc.vector.tensor_scalar(out=linf2[:], in0=linf[:],
                                    scalar1=-(WP + 1.0) * (FB - PADR),
                                    scalar2=None, op0=ALU.add)
            lin16 = gpool.tile([P, NG], i16)
            nc.vector.tensor_copy(out=lin16[:], in_=linf2[:])

            # bilinear weight products, 4-lane interleave matching P4 lanes
            wpre_cat = gpool.tile([P, 4 * NG], bf16)
            wv = wpre_cat[:].rearrange("p (k q j) -> p k q j", k=K, q=Q, j=4)
            omly3 = omly[:].rearrange("p (k q) -> p k q", k=K, q=Q)
            ly3 = ly[:].rearrange("p (k q) -> p k q", k=K, q=Q)
            omlx3 = omlx[:].rearrange("p (k q) -> p k q", k=K, q=Q)
            lx3 = lx[:].rearrange("p (k q) -> p k q", k=K, q=Q)
            nc.vector.tensor_tensor(out=wv[:, :, :, 0], in0=omly3, in1=omlx3,
                                    op=ALU.mult)  # w00 (A)
            nc.vector.tensor_tensor(out=wv[:, :, :, 1], in0=ly3, in1=omlx3,
                                    op=ALU.mult)  # w10 (C)
            nc.vector.tensor_tensor(out=wv[:, :, :, 2], in0=omly3, in1=lx3,
                                    op=ALU.mult)  # w01 (B)
            nc.vector.tensor_tensor(out=wv[:, :, :, 3], in0=ly3, in1=lx3,
                                    op=ALU.mult)  # w11 (D)

            # stage per-tap rows: wrow[k, (q*128+Pp)*4 + j] = wpre_cat[Pp, kqj]
            wrow_v = wrow[:].rearrange("k (q p j) -> p k q j", k=K, q=Q, p=P, j=4)
            wpre_v = wpre_cat[:].rearrange("p (k q j) -> p k q j", k=K, q=Q, j=4)
            for k in range(K):
                eng = nc.sync if k % 2 == 0 else nc.scalar
                eng.dma_start(out=wrow_v[:, k], in_=wpre_v[:, k])

            # gather index tensor: wrapped-16 layout for ap_gather
            NI = 8 * K * Q  # 2304 idx-cols total (256 per tap)
            idxw = gpool.tile([P, NI], i16)
            for u in range(8):
                eng = nc.sync if u % 2 == 0 else nc.scalar
                eng.dma_start(
                    out=idxw[0:16, :].rearrange(
                        "p (k q u) -> p k q u", k=K, q=Q, u=8)[:, :, :, u],
                    in_=lin16[16 * u: 16 * u + 16, :].rearrange(
                        "p (k q) -> p k q", k=K, q=Q),
                )
            nc.sync.dma_start(out=idxw[16:32, :], in_=idxw[0:16, :])
            nc.scalar.dma_start(out=idxw[32:64, :], in_=idxw[0:32, :])
            nc.sync.dma_start(out=idxw[64:128, :], in_=idxw[0:64, :])

            # ---------------- main loop: eighths x taps ----------------
            rr = 0
            for e in range(NE8):
                psum = pspool.tile([P, 4 * NQ8], mybir.dt.float32, tag="ps")
                for k in range(K):
                    wb = wpool.tile([P, 4 * NQ8], bf16, tag="wb")
                    sl = slice(e * 4 * NQ8, (e + 1) * 4 * NQ8)
                    eng = nc.sync if rr % 2 == 0 else nc.scalar
                    rr += 1
                    eng.dma_start(
                        out=wb[:],
                        in_=wrow[k: k + 1, sl].to_broadcast((P, 4 * NQ8)))
                    i0 = k * (8 * Q) + e * QI8
                    g4 = gapool.tile([P, 2 * NQ8], f32, tag="g")
                    nc.gpsimd.ap_gather(
                        g4[:], P4[:], idxw[:, i0: i0 + QI8],
                        channels=P, num_elems=NE, d=2, num_idxs=NQ8)
                    m = mpool.tile([P, 4 * NQ8], bf16, tag="m")
                    nc.vector.tensor_tensor(
                        out=m[:], in0=g4[:].bitcast(bf16), in1=wb[:],
                        op=ALU.mult)
                    lhsT = wmat[:, k * COUT: (k + 1) * COUT]
                    for bk in range(4):
                        c0 = bk * 512
                        nc.tensor.matmul(
                            psum[:, c0: c0 + 512], lhsT, m[:, c0: c0 + 512],
                            start=(k == 0), stop=(k == K - 1),
                            skip_group_check=True)
                # tail: sum 4 corner lanes + bias -> fp32 out
                pv = psum[:].rearrange("p (n j) -> p n j", j=4)
                t = opool.tile([P, NQ8], f32, tag="t")
                nc.vector.tensor_scalar(
                    out=t[:], in0=pv[:, :, 0],
                    scalar1=bias[:, 0:1], scalar2=None, op0=ALU.add)
                t2 = opool.tile([P, NQ8], f32, tag="t2")
                nc.vector.tensor_tensor(out=t2[:], in0=t[:], in1=pv[:, :, 1],
                                        op=ALU.add)
                t3 = opool.tile([P, NQ8], f32, tag="t3")
                nc.vector.tensor_tensor(out=t3[:], in0=t2[:], in1=pv[:, :, 2],
                                        op=ALU.add)
                o = opool.tile([P, NQ8], f32, tag="o")
                nc.vector.tensor_tensor(out=o[:], in0=t3[:], in1=pv[:, :, 3],
                                        op=ALU.add)
                nc.gpsimd.dma_start(
                    out=out_d.ap()[:, e * NQ8: (e + 1) * NQ8], in_=o[:])

    nc.compile()
    return nc


_NC_CACHE = None


def _host_inputs(x, offset, weight, bias):
    """Per-core input maps (core b <- batch b) + replicated constants."""
    wq = np.ascontiguousarray(weight, np.float32)  # [COUT, CIN, KH, KW]
    # wmat[c, k*COUT + o] = weight[o, c, ky, kx]
    wmat = wq.reshape(COUT, CIN, K).transpose(1, 2, 0).reshape(CIN, K * COUT)
    wmat = np.ascontiguousarray(wmat).astype(ml_dtypes.bfloat16)
    bias_h = np.ascontiguousarray(bias, np.float32).reshape(P, 1)
    # hob[Pp, k*Q+q] = ho(p) - 1 + ky,  wob = wo(p) - 1 + kx,  p = q*128 + Pp
    p_of = (np.arange(Q)[:, None] * P + np.arange(P)[None, :])  # [Q, P]
    ho = (p_of // WO).astype(np.float32)
    wo = (p_of % WO).astype(np.float32)
    hobwob = np.empty((P, 2 * K * Q), np.float32)
    for k in range(K):
        hobwob[:, k * Q: (k + 1) * Q] = (ho + (k // 3 - 1)).T
        hobwob[:, K * Q + k * Q: K * Q + (k + 1) * Q] = (wo + (k % 3 - 1)).T
    in_maps = []
    for b in range(B):
        xb = np.ascontiguousarray(x[b], np.float32).reshape(P, H, W)
        XP = np.zeros((P, NXP), np.float32)
        XP[:, 0:NE].reshape(P, HP, WP)[:, PADR:PADR + H, PADR:PADR + W] = xb
        p4h = np.empty((P, 4 * NE), ml_dtypes.bfloat16)
        p4h[:, 0::4] = XP[:, 0:NE]
        p4h[:, 1::4] = XP[:, WP:NE + WP]
        p4h[:, 2::4] = XP[:, 1:NE + 1]
        p4h[:, 3::4] = XP[:, WP + 1:NE + WP + 1]
        p4 = p4h.view(np.float32)  # [P, 2*NE]
        offb = np.ascontiguousarray(offset[b], np.float32).reshape(2 * K, NPOS)
        offt = np.empty((P, 2 * K * Q), np.float32)
        for k in range(K):
            offt[:, k * Q: (k + 1) * Q] = offb[2 * k].reshape(Q, P).T
            offt[:, K * Q + k * Q: K * Q + (k + 1) * Q] = \
                offb[2 * k + 1].reshape(Q, P).T
        in_maps.append({
            "p4": p4,
            "offt": offt,
            "hobwob": hobwob,
            "wmat": wmat,
            "bias": bias_h,
        })
    return in_maps


def kernel(x, offset, weight, bias):
    global _NC_CACHE
    from concourse.bass_utils import run_bass_kernel_spmd

    if _NC_CACHE is None:
        _NC_CACHE = _build_kernel()
    nc = _NC_CACHE
    in_maps = _host_inputs(x, offset, weight, bias)
    res = run_bass_kernel_spmd(nc, in_maps, list(range(B)))
    out = np.stack([res.results[b]["out"].reshape(COUT, HO, WO) for b in range(B)])
    return out.astype(np.float32)


if __name__ == "__main__":
    import sys
    d = np.load("/tmp/inputs.npz")
    if len(sys.argv) > 1 and sys.argv[1] == "sim":
        from concourse.bass_interp import CoreSim
        nc = _build_kernel()
        in_maps = _host_inputs(d["x"], d["offset"], d["weight"], d["bias"])
        sim = CoreSim(nc)
        for kk, vv in in_maps[0].items():
            sim.tensor(kk)[:] = vv
        sim.simulate()
        out = np.asarray(sim.tensor("out")).reshape(1, COUT, HO, WO)
        exp = np.load("/tmp/expected.npy")[0:1]
    else:
        out = kernel(d["x"], d["offset"], d["weight"], d["bias"])
        exp = np.load("/tmp/expected.npy")
    err = np.abs(out - exp)
    print("rel l2:", np.linalg.norm(out - exp) / np.linalg.norm(exp))
    print("absmax rel:", err.max() / np.abs(exp).max())
